# revision 10
# baseline (speedup 1.0000x reference)
"""Binarized 3x3 conv block (sign(W) conv + batch-stat BN + ReLU + 2x2 maxpool)
on 8 Trainium2 NeuronCores.

Strategy: data-parallel over batch (4 images/core). The 3x3 stride-1 pad-1
conv is 18 accumulating matmuls per output row-block (2 input-channel chunks
x 9 taps) with sign(W) as the stationary operand in bf16. Per-channel sum /
sum-of-squares are accumulated during PSUM eviction (ScalarE accum_out), and
2x2 max- AND min-pools of the raw conv output are computed at the same time
(maxpool commutes with the monotone BN+ReLU: the answer is
relu(max(s*maxp, s*minp) + b) for either sign of s). The output-channel
chunks are processed serially so chunk 0's cross-core stats AllReduce and
BN-apply hide under chunk 1's conv; only chunk 1's collective + apply are
exposed at the tail.
"""

import numpy as np
import ml_dtypes

_NCORES = 8
_B, _C, _H, _W = 32, 256, 56, 56
_BS = _B // _NCORES          # images per core
_PH, _PW = _H + 2, _W + 2    # padded input
_OH, _OW = _H // 2, _W // 2  # pooled output
_EPS = 1e-5
_NSTAT = float(_B * _H * _W)  # elements per channel in the BN stats
_RB = 7                       # row blocks per image (8 output rows each)
_BF16 = ml_dtypes.bfloat16

_CACHE: dict = {}


def _build():
    import concourse.bacc as bacc
    import concourse.mybir as mybir
    import concourse.tile as tile

    f32 = mybir.dt.float32
    bf16 = mybir.dt.bfloat16
    AF = mybir.ActivationFunctionType
    AX = mybir.AxisListType
    OP = mybir.AluOpType

    nc = bacc.Bacc("TRN2", target_bir_lowering=False, debug=False,
                   num_devices=_NCORES)
    xp_d = nc.dram_tensor("xp", [_BS, _C, _PH, _PW], bf16, kind="ExternalInput")
    w_d = nc.dram_tensor("wt", [2, 128, 9, _C], bf16, kind="ExternalInput")
    g_d = nc.dram_tensor("gm", [2, 128, 1], f32, kind="ExternalInput")
    bt_d = nc.dram_tensor("bt", [2, 128, 1], f32, kind="ExternalInput")
    out_d = nc.dram_tensor("out", [_BS, _C, _OH, _OW], f32, kind="ExternalOutput")

    with tile.TileContext(nc) as tc:
        with (
            tc.tile_pool(name="persist", bufs=1) as keep,
            tc.tile_pool(name="evict", bufs=3) as evp,
            tc.tile_pool(name="acc", bufs=8, space="PSUM") as psp,
            tc.tile_pool(name="dram", bufs=1, space="DRAM") as dpool,
        ):
            # ---- weights first (vector queue), then all images (sync queue)
            w_sb = [keep.tile([128, 9, _C], bf16, tag=f"w{c}", name=f"w{c}")
                    for c in range(2)]
            for c in range(2):
                nc.scalar.dma_start(w_sb[c][:], w_d[c])
            xt = [[keep.tile([128, _PH, _PW], bf16, tag=f"x{i}_{c}",
                             name=f"x{i}_{c}") for c in range(2)]
                  for i in range(_BS)]
            for i in range(_BS):
                for c in range(2):
                    eng = nc.sync if c == 0 else nc.gpsimd
                    eng.dma_start(xt[i][c][:], xp_d[i, c * 128:(c + 1) * 128])
            gm_sb = [keep.tile([128, 1], f32, tag=f"gm{c}", name=f"gm{c}")
                     for c in range(2)]
            bt_sb = [keep.tile([128, 1], f32, tag=f"bt{c}", name=f"bt{c}")
                     for c in range(2)]
            for c in range(2):
                nc.scalar.dma_start(gm_sb[c][:], g_d[c])
                nc.scalar.dma_start(bt_sb[c][:], bt_d[c])
            eps = keep.tile([128, 1], f32, tag="eps", name="eps")
            nc.gpsimd.memset(eps[:], _EPS)

            sumc = [keep.tile([128, _BS * _RB], f32, tag=f"sum{c}", name=f"sum{c}")
                    for c in range(2)]
            sqc = [keep.tile([128, _BS * _RB], f32, tag=f"sq{c}", name=f"sq{c}")
                   for c in range(2)]
            pmax = [[keep.tile([128, _OH, _OW], bf16, tag=f"pmax{i}_{c}",
                               name=f"pmax{i}_{c}") for c in range(2)]
                    for i in range(_BS)]
            pmin = [[keep.tile([128, _OH, _OW], bf16, tag=f"pmin{i}_{c}",
                               name=f"pmin{i}_{c}") for c in range(2)]
                    for i in range(_BS)]
            gstats = [keep.tile([128, 2], f32, tag=f"gstats{c}", name=f"gstats{c}")
                      for c in range(2)]

            # ---- conv + on-the-fly stats/pools, then the chunk's AllReduce ----
            for ch in range(2):            # output channel chunk (serialized)
                for img in range(_BS):
                    for rb in range(_RB):  # 8 output rows per block
                        ps = psp.tile([128, 8, _W], f32, tag="acc",
                                      name=f"acc{ch}_{img}_{rb}")
                        k = 0
                        for cic in range(2):
                            for kh in range(3):
                                for kw in range(3):
                                    lhsT = w_sb[cic][:, kh * 3 + kw,
                                                     ch * 128:(ch + 1) * 128]
                                    rhs = xt[img][cic][:,
                                                       rb * 8 + kh: rb * 8 + kh + 8,
                                                       kw: kw + _W]
                                    nc.tensor.matmul(ps[:], lhsT, rhs,
                                                     start=(k == 0),
                                                     stop=(k == 17))
                                    k += 1
                        col = img * _RB + rb
                        last = (img == _BS - 1 and rb == _RB - 1)
                        ybf = evp.tile([128, 8, _W], bf16, tag="ybf",
                                       name=f"ybf{ch}_{col}")
                        if last:
                            # keep the collective-trigger path short: channel
                            # sum on DVE straight from PSUM, square on ScalarE
                            # in parallel; the ybf copy only feeds the pools
                            nc.scalar.activation(ybf[:], ps[:], AF.Copy)
                            nc.vector.reduce_sum(sumc[ch][:, col:col + 1],
                                                 ps[:], axis=AX.XY)
                        else:
                            nc.scalar.activation(
                                ybf[:], ps[:], AF.Copy,
                                accum_out=sumc[ch][:, col:col + 1])
                        ysq = evp.tile([128, 8, _W], bf16, tag="ysq",
                                       name=f"ysq{ch}_{col}")
                        nc.scalar.activation(
                            ysq[:], ps[:], AF.Square,
                            accum_out=sqc[ch][:, col:col + 1])
                        a = ybf[:, 0:8:2, 0:_W:2]
                        b = ybf[:, 0:8:2, 1:_W:2]
                        cc = ybf[:, 1:8:2, 0:_W:2]
                        d = ybf[:, 1:8:2, 1:_W:2]
                        t1 = evp.tile([128, 4, _OW], bf16, tag="t1",
                                      name=f"t1_{ch}_{col}")
                        t2 = evp.tile([128, 4, _OW], bf16, tag="t2",
                                      name=f"t2_{ch}_{col}")
                        nc.vector.tensor_max(t1[:], a, b)
                        nc.vector.tensor_max(t2[:], cc, d)
                        nc.vector.tensor_max(
                            pmax[img][ch][:, rb * 4:(rb + 1) * 4, :], t1[:], t2[:])
                        t3 = evp.tile([128, 4, _OW], bf16, tag="t3",
                                      name=f"t3_{ch}_{col}")
                        t4 = evp.tile([128, 4, _OW], bf16, tag="t4",
                                      name=f"t4_{ch}_{col}")
                        nc.vector.tensor_tensor(t3[:], a, b, op=OP.min)
                        nc.vector.tensor_tensor(t4[:], cc, d, op=OP.min)
                        nc.vector.tensor_tensor(
                            pmin[img][ch][:, rb * 4:(rb + 1) * 4, :], t3[:], t4[:],
                            op=OP.min)

                # chunk's global stats: [128,2] AllReduce across the 8 cores
                stats = keep.tile([128, 2], f32, tag=f"stats{ch}",
                                  name=f"stats{ch}")
                nc.vector.reduce_sum(stats[:, 0:1], sumc[ch][:], axis=AX.X)
                nc.vector.reduce_sum(stats[:, 1:2], sqc[ch][:], axis=AX.X)
                cc_in = dpool.tile([128, 2], f32, tag=f"ccin{ch}",
                                   name=f"ccin{ch}")
                cc_out = dpool.tile([128, 2], f32, tag=f"ccout{ch}",
                                    name=f"ccout{ch}")
                nc.sync.dma_start(cc_in[:], stats[:])
                nc.gpsimd.collective_compute(
                    "AllReduce", OP.add,
                    replica_groups=[list(range(_NCORES))],
                    ins=[cc_in.opt()], outs=[cc_out.opt()])
                nc.sync.dma_start(gstats[ch][:], cc_out[:])

            # ---- per-chunk scale/bias + BN/ReLU apply + store ----
            # (emitted after both conv chunks: chunk 0's instructions have no
            # outstanding deps by the time engines reach them, so they fill
            # the window while chunk 1's collective runs)
            for ch in range(2):
                meanq = keep.tile([128, 2], f32, tag=f"meanq{ch}",
                                  name=f"meanq{ch}")
                m2 = keep.tile([128, 1], f32, tag=f"m2{ch}", name=f"m2{ch}")
                var = keep.tile([128, 1], f32, tag=f"var{ch}", name=f"var{ch}")
                sd = keep.tile([128, 1], f32, tag=f"sd{ch}", name=f"sd{ch}")
                inv = keep.tile([128, 1], f32, tag=f"inv{ch}", name=f"inv{ch}")
                s = keep.tile([128, 1], f32, tag=f"s{ch}", name=f"s{ch}")
                ms = keep.tile([128, 1], f32, tag=f"ms{ch}", name=f"ms{ch}")
                bb = keep.tile([128, 1], f32, tag=f"bb{ch}", name=f"bb{ch}")
                nc.scalar.mul(meanq[:], gstats[ch][:], 1.0 / _NSTAT)
                nc.vector.tensor_mul(m2[:], meanq[:, 0:1], meanq[:, 0:1])
                nc.vector.tensor_sub(var[:], meanq[:, 1:2], m2[:])
                nc.scalar.activation(sd[:], var[:], AF.Sqrt, bias=eps[:])
                nc.vector.reciprocal(inv[:], sd[:])
                nc.vector.tensor_mul(s[:], gm_sb[ch][:], inv[:])
                nc.vector.tensor_mul(ms[:], meanq[:, 0:1], s[:])
                nc.vector.tensor_sub(bb[:], bt_sb[ch][:], ms[:])

                for img in range(_BS):
                    u = evp.tile([128, _OH, _OW], bf16, tag="u",
                                 name=f"u{ch}_{img}")
                    v = evp.tile([128, _OH, _OW], bf16, tag="v",
                                 name=f"v{ch}_{img}")
                    m = evp.tile([128, _OH, _OW], bf16, tag="m",
                                 name=f"m{ch}_{img}")
                    nc.vector.tensor_scalar_mul(u[:], pmax[img][ch][:], s[:])
                    nc.vector.tensor_scalar_mul(v[:], pmin[img][ch][:], s[:])
                    nc.vector.tensor_max(m[:], u[:], v[:])
                    res = evp.tile([128, _OH, _OW], f32, tag="res",
                                   name=f"res{ch}_{img}")
                    nc.scalar.activation(res[:], m[:], AF.Relu, bias=bb[:])
                    eng = nc.sync if img % 2 == 0 else nc.gpsimd
                    eng.dma_start(out_d[img, ch * 128:(ch + 1) * 128], res[:])

    nc.compile()
    return nc


def _prep_inputs(x, W, gamma, beta):
    x = np.asarray(x, dtype=np.float32)
    W = np.asarray(W, dtype=np.float32)
    gamma = np.asarray(gamma, dtype=np.float32)
    beta = np.asarray(beta, dtype=np.float32)

    # sign-binarized weights, laid out [ci_chunk, ci_in_chunk, tap, co] bf16
    wb = np.sign(W)                                    # [co, ci, 3, 3]
    wt = wb.transpose(1, 2, 3, 0).reshape(2, 128, 9, _C)
    wt = np.ascontiguousarray(wt).astype(_BF16)

    # zero-padded, bf16-cast input
    xp = np.zeros((_B, _C, _PH, _PW), dtype=_BF16)
    xp[:, :, 1:_H + 1, 1:_W + 1] = x.astype(_BF16)

    gm = np.ascontiguousarray(gamma.reshape(2, 128, 1))
    bt = np.ascontiguousarray(beta.reshape(2, 128, 1))

    in_maps = []
    for core in range(_NCORES):
        in_maps.append({
            "xp": np.ascontiguousarray(xp[core * _BS:(core + 1) * _BS]),
            "wt": wt,
            "gm": gm,
            "bt": bt,
        })
    return in_maps


def _run(x, W, gamma, beta, trace=False):
    from concourse.bass_utils import run_bass_kernel_spmd

    if "nc" not in _CACHE:
        _CACHE["nc"] = _build()
    nc = _CACHE["nc"]
    in_maps = _prep_inputs(x, W, gamma, beta)
    res = run_bass_kernel_spmd(nc, in_maps, core_ids=list(range(_NCORES)),
                               trace=trace)
    out = np.concatenate([res.results[c]["out"] for c in range(_NCORES)], axis=0)
    return np.ascontiguousarray(out.astype(np.float32)), res


def kernel(x, W, gamma, beta):
    out, _ = _run(x, W, gamma, beta, trace=False)
    return out


# revision 12
# speedup vs baseline: 1.0277x; 1.0277x over previous
"""Binarized 3x3 conv block (sign(W) conv + batch-stat BN + ReLU + 2x2 maxpool)
on 8 Trainium2 NeuronCores.

Strategy: data-parallel over batch (4 images/core). The 3x3 stride-1 pad-1
conv is 18 accumulating matmuls per output row-block (2 input-channel chunks
x 9 taps) with sign(W) as the stationary operand in bf16. Per-channel sum /
sum-of-squares are accumulated during PSUM eviction (ScalarE accum_out), and
2x2 max- AND min-pools of the raw conv output are computed at the same time
(maxpool commutes with the monotone BN+ReLU: the answer is
relu(max(s*maxp, s*minp) + b) for either sign of s). The output-channel
chunks are processed serially so chunk 0's cross-core stats AllReduce and
BN-apply hide under chunk 1's conv; only chunk 1's collective + apply are
exposed at the tail.
"""

import numpy as np
import ml_dtypes

_NCORES = 8
_B, _C, _H, _W = 32, 256, 56, 56
_BS = _B // _NCORES          # images per core
_PH, _PW = _H + 2, _W + 2    # padded input
_OH, _OW = _H // 2, _W // 2  # pooled output
_EPS = 1e-5
_NSTAT = float(_B * _H * _W)  # elements per channel in the BN stats
_RB = 7                       # row blocks per image (8 output rows each)
_BF16 = ml_dtypes.bfloat16

_CACHE: dict = {}


def _build():
    import concourse.bacc as bacc
    import concourse.mybir as mybir
    import concourse.tile as tile

    f32 = mybir.dt.float32
    bf16 = mybir.dt.bfloat16
    AF = mybir.ActivationFunctionType
    AX = mybir.AxisListType
    OP = mybir.AluOpType

    nc = bacc.Bacc("TRN2", target_bir_lowering=False, debug=False,
                   num_devices=_NCORES)
    xp_d = nc.dram_tensor("xp", [_BS, _C, _PH, _PW], bf16, kind="ExternalInput")
    w_d = nc.dram_tensor("wt", [2, 128, 9, _C], bf16, kind="ExternalInput")
    g_d = nc.dram_tensor("gm", [2, 128, 1], f32, kind="ExternalInput")
    bt_d = nc.dram_tensor("bt", [2, 128, 1], f32, kind="ExternalInput")
    out_d = nc.dram_tensor("out", [_BS, _C, _OH, _OW], f32, kind="ExternalOutput")

    with tile.TileContext(nc) as tc:
        with (
            tc.tile_pool(name="persist", bufs=1) as keep,
            tc.tile_pool(name="evict", bufs=3) as evp,
            tc.tile_pool(name="acc", bufs=8, space="PSUM") as psp,
            tc.tile_pool(name="dram", bufs=1, space="DRAM") as dpool,
        ):
            # ---- weights first (vector queue), then all images (sync queue)
            w_sb = [keep.tile([128, 9, _C], bf16, tag=f"w{c}", name=f"w{c}")
                    for c in range(2)]
            for c in range(2):
                nc.scalar.dma_start(w_sb[c][:], w_d[c])
            xt = [[None, None]] + \
                 [[keep.tile([128, _PH, _PW], bf16, tag=f"x{i}_{c}",
                             name=f"x{i}_{c}") for c in range(2)]
                  for i in range(1, _BS)]
            # img0 arrives as two row-chunks per ci chunk (A: rows 0..33 for
            # row-blocks 0-3, B: rows 32..57 for 4-6) so the PE starts early
            x0a = [keep.tile([128, 34, _PW], bf16, tag=f"x0a_{c}",
                             name=f"x0a_{c}") for c in range(2)]
            x0b = [keep.tile([128, 26, _PW], bf16, tag=f"x0b_{c}",
                             name=f"x0b_{c}") for c in range(2)]
            for c in range(2):
                eng = nc.sync if c == 0 else nc.gpsimd
                eng.dma_start(x0a[c][:], xp_d[0, c * 128:(c + 1) * 128, 0:34])
                eng.dma_start(x0b[c][:], xp_d[0, c * 128:(c + 1) * 128, 32:58])
            for i in range(1, _BS):
                for c in range(2):
                    eng = nc.sync if c == 0 else nc.gpsimd
                    eng.dma_start(xt[i][c][:], xp_d[i, c * 128:(c + 1) * 128])
            gm_sb = [keep.tile([128, 1], f32, tag=f"gm{c}", name=f"gm{c}")
                     for c in range(2)]
            bt_sb = [keep.tile([128, 1], f32, tag=f"bt{c}", name=f"bt{c}")
                     for c in range(2)]
            for c in range(2):
                nc.scalar.dma_start(gm_sb[c][:], g_d[c])
                nc.scalar.dma_start(bt_sb[c][:], bt_d[c])
            eps = keep.tile([128, 1], f32, tag="eps", name="eps")
            nc.gpsimd.memset(eps[:], _EPS)

            sumc = [keep.tile([128, _BS * _RB], f32, tag=f"sum{c}", name=f"sum{c}")
                    for c in range(2)]
            sqc = [keep.tile([128, _BS * _RB], f32, tag=f"sq{c}", name=f"sq{c}")
                   for c in range(2)]
            pmax = [[keep.tile([128, _OH, _OW], bf16, tag=f"pmax{i}_{c}",
                               name=f"pmax{i}_{c}") for c in range(2)]
                    for i in range(_BS)]
            pmin = [[keep.tile([128, _OH, _OW], bf16, tag=f"pmin{i}_{c}",
                               name=f"pmin{i}_{c}") for c in range(2)]
                    for i in range(_BS)]
            gstats = [keep.tile([128, 2], f32, tag=f"gstats{c}", name=f"gstats{c}")
                      for c in range(2)]

            # ---- conv + on-the-fly stats/pools, then the chunk's AllReduce ----
            for ch in range(2):            # output channel chunk (serialized)
                for img in range(_BS):
                    for rb in range(_RB):  # 8 output rows per block
                        ps = psp.tile([128, 8, _W], f32, tag="acc",
                                      name=f"acc{ch}_{img}_{rb}")
                        k = 0
                        for cic in range(2):
                            for kh in range(3):
                                for kw in range(3):
                                    lhsT = w_sb[cic][:, kh * 3 + kw,
                                                     ch * 128:(ch + 1) * 128]
                                    if img == 0:
                                        r0 = rb * 8 + kh
                                        if rb < 4:
                                            rhs = x0a[cic][:, r0: r0 + 8,
                                                           kw: kw + _W]
                                        else:
                                            rhs = x0b[cic][:, r0 - 32: r0 - 24,
                                                           kw: kw + _W]
                                    else:
                                        rhs = xt[img][cic][:,
                                                           rb * 8 + kh: rb * 8 + kh + 8,
                                                           kw: kw + _W]
                                    nc.tensor.matmul(ps[:], lhsT, rhs,
                                                     start=(k == 0),
                                                     stop=(k == 17))
                                    k += 1
                        col = img * _RB + rb
                        last = (img == _BS - 1 and rb == _RB - 1)
                        ybf = evp.tile([128, 8, _W], bf16, tag="ybf",
                                       name=f"ybf{ch}_{col}")
                        if last:
                            # keep the collective-trigger path short: channel
                            # sum on DVE straight from PSUM, square on ScalarE
                            # in parallel; the ybf copy only feeds the pools
                            nc.scalar.activation(ybf[:], ps[:], AF.Copy)
                            nc.vector.reduce_sum(sumc[ch][:, col:col + 1],
                                                 ps[:], axis=AX.XY)
                        else:
                            nc.scalar.activation(
                                ybf[:], ps[:], AF.Copy,
                                accum_out=sumc[ch][:, col:col + 1])
                        ysq = evp.tile([128, 8, _W], bf16, tag="ysq",
                                       name=f"ysq{ch}_{col}")
                        nc.scalar.activation(
                            ysq[:], ps[:], AF.Square,
                            accum_out=sqc[ch][:, col:col + 1])
                        a = ybf[:, 0:8:2, 0:_W:2]
                        b = ybf[:, 0:8:2, 1:_W:2]
                        cc = ybf[:, 1:8:2, 0:_W:2]
                        d = ybf[:, 1:8:2, 1:_W:2]
                        t1 = evp.tile([128, 4, _OW], bf16, tag="t1",
                                      name=f"t1_{ch}_{col}")
                        t2 = evp.tile([128, 4, _OW], bf16, tag="t2",
                                      name=f"t2_{ch}_{col}")
                        nc.vector.tensor_max(t1[:], a, b)
                        nc.vector.tensor_max(t2[:], cc, d)
                        nc.vector.tensor_max(
                            pmax[img][ch][:, rb * 4:(rb + 1) * 4, :], t1[:], t2[:])
                        t3 = evp.tile([128, 4, _OW], bf16, tag="t3",
                                      name=f"t3_{ch}_{col}")
                        t4 = evp.tile([128, 4, _OW], bf16, tag="t4",
                                      name=f"t4_{ch}_{col}")
                        nc.vector.tensor_tensor(t3[:], a, b, op=OP.min)
                        nc.vector.tensor_tensor(t4[:], cc, d, op=OP.min)
                        nc.vector.tensor_tensor(
                            pmin[img][ch][:, rb * 4:(rb + 1) * 4, :], t3[:], t4[:],
                            op=OP.min)

                # chunk's global stats: [128,2] AllReduce across the 8 cores
                stats = keep.tile([128, 2], f32, tag=f"stats{ch}",
                                  name=f"stats{ch}")
                nc.vector.reduce_sum(stats[:, 0:1], sumc[ch][:], axis=AX.X)
                nc.vector.reduce_sum(stats[:, 1:2], sqc[ch][:], axis=AX.X)
                cc_in = dpool.tile([128, 2], f32, tag=f"ccin{ch}",
                                   name=f"ccin{ch}")
                cc_out = dpool.tile([128, 2], f32, tag=f"ccout{ch}",
                                    name=f"ccout{ch}")
                nc.sync.dma_start(cc_in[:], stats[:])
                nc.gpsimd.collective_compute(
                    "AllReduce", OP.add,
                    replica_groups=[list(range(_NCORES))],
                    ins=[cc_in.opt()], outs=[cc_out.opt()])
                nc.sync.dma_start(gstats[ch][:], cc_out[:])

            # ---- per-chunk scale/bias + BN/ReLU apply + store ----
            # (emitted after both conv chunks: chunk 0's instructions have no
            # outstanding deps by the time engines reach them, so they fill
            # the window while chunk 1's collective runs)
            for ch in range(2):
                meanq = keep.tile([128, 2], f32, tag=f"meanq{ch}",
                                  name=f"meanq{ch}")
                m2 = keep.tile([128, 1], f32, tag=f"m2{ch}", name=f"m2{ch}")
                var = keep.tile([128, 1], f32, tag=f"var{ch}", name=f"var{ch}")
                sd = keep.tile([128, 1], f32, tag=f"sd{ch}", name=f"sd{ch}")
                inv = keep.tile([128, 1], f32, tag=f"inv{ch}", name=f"inv{ch}")
                s = keep.tile([128, 1], f32, tag=f"s{ch}", name=f"s{ch}")
                ms = keep.tile([128, 1], f32, tag=f"ms{ch}", name=f"ms{ch}")
                bb = keep.tile([128, 1], f32, tag=f"bb{ch}", name=f"bb{ch}")
                nc.scalar.mul(meanq[:], gstats[ch][:], 1.0 / _NSTAT)
                nc.vector.tensor_mul(m2[:], meanq[:, 0:1], meanq[:, 0:1])
                nc.vector.tensor_sub(var[:], meanq[:, 1:2], m2[:])
                nc.scalar.activation(sd[:], var[:], AF.Sqrt, bias=eps[:])
                nc.vector.reciprocal(inv[:], sd[:])
                nc.vector.tensor_mul(s[:], gm_sb[ch][:], inv[:])
                nc.vector.tensor_mul(ms[:], meanq[:, 0:1], s[:])
                nc.vector.tensor_sub(bb[:], bt_sb[ch][:], ms[:])

                for img in range(_BS):
                    u = evp.tile([128, _OH, _OW], bf16, tag="u",
                                 name=f"u{ch}_{img}")
                    v = evp.tile([128, _OH, _OW], bf16, tag="v",
                                 name=f"v{ch}_{img}")
                    m = evp.tile([128, _OH, _OW], bf16, tag="m",
                                 name=f"m{ch}_{img}")
                    nc.vector.tensor_scalar_mul(u[:], pmax[img][ch][:], s[:])
                    nc.vector.tensor_scalar_mul(v[:], pmin[img][ch][:], s[:])
                    nc.vector.tensor_max(m[:], u[:], v[:])
                    res = evp.tile([128, _OH, _OW], f32, tag="res",
                                   name=f"res{ch}_{img}")
                    nc.scalar.activation(res[:], m[:], AF.Relu, bias=bb[:])
                    eng = nc.sync if img % 2 == 0 else nc.gpsimd
                    eng.dma_start(out_d[img, ch * 128:(ch + 1) * 128], res[:])

    nc.compile()
    return nc


def _prep_inputs(x, W, gamma, beta):
    x = np.asarray(x, dtype=np.float32)
    W = np.asarray(W, dtype=np.float32)
    gamma = np.asarray(gamma, dtype=np.float32)
    beta = np.asarray(beta, dtype=np.float32)

    # sign-binarized weights, laid out [ci_chunk, ci_in_chunk, tap, co] bf16
    wb = np.sign(W)                                    # [co, ci, 3, 3]
    wt = wb.transpose(1, 2, 3, 0).reshape(2, 128, 9, _C)
    wt = np.ascontiguousarray(wt).astype(_BF16)

    # zero-padded, bf16-cast input
    xp = np.zeros((_B, _C, _PH, _PW), dtype=_BF16)
    xp[:, :, 1:_H + 1, 1:_W + 1] = x.astype(_BF16)

    gm = np.ascontiguousarray(gamma.reshape(2, 128, 1))
    bt = np.ascontiguousarray(beta.reshape(2, 128, 1))

    in_maps = []
    for core in range(_NCORES):
        in_maps.append({
            "xp": np.ascontiguousarray(xp[core * _BS:(core + 1) * _BS]),
            "wt": wt,
            "gm": gm,
            "bt": bt,
        })
    return in_maps


def _run(x, W, gamma, beta, trace=False):
    from concourse.bass_utils import run_bass_kernel_spmd

    if "nc" not in _CACHE:
        _CACHE["nc"] = _build()
    nc = _CACHE["nc"]
    in_maps = _prep_inputs(x, W, gamma, beta)
    res = run_bass_kernel_spmd(nc, in_maps, core_ids=list(range(_NCORES)),
                               trace=trace)
    out = np.concatenate([res.results[c]["out"] for c in range(_NCORES)], axis=0)
    return np.ascontiguousarray(out.astype(np.float32)), res


def kernel(x, W, gamma, beta):
    out, _ = _run(x, W, gamma, beta, trace=False)
    return out
